# revision 1
# baseline (speedup 1.0000x reference)
"""Trainium2 Bass kernel for nn_HBlock (dense transformer block, GQA + softcap + relu^2 MLP).

Sharding: 8 cores = DP(batch=2) x TP(4 kv-head groups). Each core computes
attention for its 4 q-heads over the full update range (uniform causal
structure), then an AllGather (per head-pair, overlapped) reshards so each
core runs o-proj/residual/MLP for its own 528-token q-slab.

All device compute is feature-major ("T-major": feature dim on partitions,
tokens on the free dim), so no activation transposes are needed anywhere
except V (33 small PE transposes). Softmax denominators come for free from
an appended ones-column on V. Matmuls run in bf16 with fp32 PSUM accum.
"""
import numpy as np
import ml_dtypes

import concourse.bass as bass
import concourse.tile as tile
from concourse import bacc, mybir
from concourse.bass import ds, ts
from concourse.bass_utils import run_bass_kernel_spmd

# problem constants (hardcoded per contract)
B, T, D = 2, 4160, 1024
H, KVH, HD = 16, 4, 64
QSTART = 2048
Q = T - QSTART            # 2112 update tokens
NG = 4                    # TP groups per batch
QS = Q // NG              # 528 q-slab per core
SOFTCAP = 15.0
EPS_RMS = 1e-6
NEG = -1e9

P = 128
DCH = D // P              # 8 feature chunks
NKT = (T + P - 1) // P    # 33 kv tiles (last = 64 wide)
QBS = [512, 512, 512, 512, 64]   # q blocks over the 2112 update tokens
FDIM = 512
QF = 264                  # q free-tile for the MLP phase (528 = 2*264)

BF16 = mybir.dt.bfloat16
F32 = mybir.dt.float32


def kvw(kt):
    return min(P, T - kt * P)


def nkv(qb):
    """number of kv tiles needed for q block qb (causal)."""
    qend = QSTART + sum(QBS[: qb + 1])
    return (qend + P - 1) // P


def build(sim=False):
    nc = bacc.Bacc("TRN2", target_bir_lowering=False, debug=False,
                   num_devices=1 if sim else 8)

    xnt_d = nc.dram_tensor("xnt", [D, T], BF16, kind="ExternalInput")
    xslab_d = nc.dram_tensor("xslab", [D, QS], F32, kind="ExternalInput")
    wq_d = nc.dram_tensor("wq", [D, 4 * HD], BF16, kind="ExternalInput")
    wkv_d = nc.dram_tensor("wkv", [D, 2 * HD], BF16, kind="ExternalInput")
    wo_d = nc.dram_tensor("wo", [D, D], BF16, kind="ExternalInput")
    wfc_d = nc.dram_tensor("wfc", [32, D, P], BF16, kind="ExternalInput")
    wproj_d = nc.dram_tensor("wproj", [DCH, 4 * D, P], BF16, kind="ExternalInput")
    mask_d = nc.dram_tensor("mask", [4, P, FDIM], F32, kind="ExternalInput")
    qoff_d = nc.dram_tensor("qoff", [1, 1], mybir.dt.int32, kind="ExternalInput")
    ones_p_d = nc.dram_tensor("ones_p", [P, 1], BF16, kind="ExternalInput")
    ones_f_d = nc.dram_tensor("ones_f", [1, P], F32, kind="ExternalInput")
    ident_d = nc.dram_tensor("ident", [P, P], BF16, kind="ExternalInput")
    out_d = nc.dram_tensor("out", [D, QS], F32, kind="ExternalOutput")

    with tile.TileContext(nc) as tc:
        with tc.tile_pool(name="res", bufs=1) as res, \
             tc.tile_pool(name="dram", bufs=1, space="DRAM") as dram, \
             nc.gpsimd.register("qr") as qr:

            # ---- resident tensors / constants ----
            wq_sb = res.tile([P, DCH, 4 * HD], BF16)
            nc.sync.dma_start(wq_sb[:], wq_d.rearrange("(c p) n -> p c n", p=P))
            wkv_sb = res.tile([P, DCH, 2 * HD], BF16)
            nc.sync.dma_start(wkv_sb[:], wkv_d.rearrange("(c p) n -> p c n", p=P))
            wo_sb = res.tile([P, DCH, D], BF16)
            nc.sync.dma_start(wo_sb[:], wo_d.rearrange("(c p) n -> p c n", p=P))
            mask_sb = res.tile([P, 4, FDIM], F32)
            nc.sync.dma_start(mask_sb[:], mask_d.rearrange("m p f -> p m f"))
            ones_p = res.tile([P, 1], BF16)
            nc.sync.dma_start(ones_p[:], ones_p_d[:])
            ones_f = res.tile([1, P], F32)
            nc.sync.dma_start(ones_f[:], ones_f_d[:])
            ident = res.tile([P, P], BF16)
            nc.sync.dma_start(ident[:], ident_d[:])
            qsb = res.tile([1, 1], mybir.dt.int32)
            nc.sync.dma_start(qsb[:], qoff_d[:])
            eps_l2 = res.tile([1, 1], F32)
            nc.vector.memset(eps_l2[:], 1e-24)
            eps_x = res.tile([1, 1], F32)
            nc.vector.memset(eps_x[:], EPS_RMS)

            khT = res.tile([P, NKT * P], BF16)     # rows 0:64 = k_hat^T, 64:128 dup
            v_aug = res.tile([P, NKT, 72], BF16)   # [kv_tile_rows, tile, 64 v + ones]
            qhT = res.tile([P, 2, Q], BF16)        # [h_even|h_odd, pair, q]
            attnT = res.tile([P, 2, Q], BF16)

            nc.vector.memset(v_aug[:], 0.0)
            nc.vector.memset(v_aug[:, :, 64:65], 1.0)

            nc.gpsimd.reg_load(qr, qsb[:1, :1])
            qoff = nc.gpsimd.snap(qr)

            gin = [dram.tile([P, Q], BF16, name=f"gin{i}") for i in range(2)]
            gout = [dram.tile([4, P, Q], BF16, name=f"gout{i}") for i in range(2)]

            # ================= Phases B+C (xnt resident only here) ==========
            xnp = tc.tile_pool(name="xnp", bufs=1)
            xnpool = xnp.__enter__()
            xnt = xnpool.tile([P, DCH, T], BF16)
            nc.sync.dma_start(xnt[:], xnt_d.rearrange("(c p) t -> p c t", p=P))

            # ================= Phase B: kv-proj, k-norm, v-transpose ========
            with tc.tile_pool(name="pbs", bufs=3) as sbB, \
                 tc.tile_pool(name="pbp", bufs=2, space="PSUM") as psB:
                nblk = (T + FDIM - 1) // FDIM
                for blk in range(nblk):
                    t0 = blk * FDIM
                    bw = min(FDIM, T - t0)
                    kv_ps = psB.tile([P, FDIM], F32, tag="kv")
                    for c in range(DCH):
                        nc.tensor.matmul(
                            kv_ps[:, :bw], wkv_sb[:, c, :], xnt[:, c, t0:t0 + bw],
                            start=(c == 0), stop=(c == DCH - 1))
                    ktmp = sbB.tile([HD, FDIM], BF16, tag="ktmp")
                    nc.vector.tensor_copy(ktmp[:, :bw], kv_ps[0:HD, :bw])
                    vtmp = sbB.tile([HD, FDIM], BF16, tag="vtmp")
                    nc.vector.tensor_copy(vtmp[:, :bw], kv_ps[HD:P, :bw])
                    # k l2-norm (over the 64-partition head dim, via ones-matmul)
                    ksq = sbB.tile([HD, FDIM], BF16, tag="ksq")
                    nc.scalar.square(ksq[:, :bw], ktmp[:, :bw])
                    ss_ps = psB.tile([1, FDIM], F32, tag="ss")
                    nc.tensor.matmul(ss_ps[:, :bw], ones_p[0:HD, :], ksq[:, :bw],
                                     start=True, stop=True)
                    sq = sbB.tile([1, FDIM], F32, tag="sq")
                    nc.scalar.activation(sq[:, :bw], ss_ps[:, :bw],
                                         mybir.ActivationFunctionType.Sqrt,
                                         bias=eps_l2[:1, :1], scale=1.0)
                    rec = sbB.tile([1, FDIM], F32, tag="rec")
                    nc.vector.reciprocal(rec[:, :bw], sq[:, :bw])
                    bc_ps = psB.tile([HD, FDIM], F32, tag="bc")
                    nc.tensor.matmul(bc_ps[:, :bw], ones_f[:, 0:HD], rec[:, :bw],
                                     start=True, stop=True)
                    nc.vector.tensor_mul(khT[0:HD, t0:t0 + bw], ktmp[:, :bw],
                                         bc_ps[:, :bw])
                    nc.vector.tensor_copy(khT[HD:P, t0:t0 + bw],
                                          khT[0:HD, t0:t0 + bw])
                    # transpose v into token-major v_aug tiles
                    for tt in range((bw + P - 1) // P):
                        kt = blk * (FDIM // P) + tt
                        tw = kvw(kt)
                        tp_ps = psB.tile([P, HD], BF16, tag="tp")
                        nc.tensor.transpose(tp_ps[0:tw, :], vtmp[:, tt * P:tt * P + tw],
                                            ident[0:HD, 0:HD])
                        nc.vector.tensor_copy(v_aug[0:tw, kt, 0:HD], tp_ps[0:tw, :])

            # ================= Phase C: q-proj + q-norm (1/8 folded) ========
            with tc.tile_pool(name="pcs", bufs=3) as sbC, \
                 tc.tile_pool(name="pcp", bufs=2, space="PSUM") as psC:
                for p in range(2):
                    for qb in range(5):
                        q0 = sum(QBS[:qb])
                        qw = QBS[qb]
                        q_ps = psC.tile([P, FDIM], F32, tag="q")
                        for c in range(DCH):
                            nc.tensor.matmul(
                                q_ps[:, :qw], wq_sb[:, c, p * P:(p + 1) * P],
                                xnt[:, c, QSTART + q0:QSTART + q0 + qw],
                                start=(c == 0), stop=(c == DCH - 1))
                        qtmp = sbC.tile([P, FDIM], BF16, tag="qtmp")
                        nc.vector.tensor_copy(qtmp[:, :qw], q_ps[:, :qw])
                        qsq = sbC.tile([P, FDIM], BF16, tag="qsq")
                        nc.scalar.square(qsq[:, :qw], qtmp[:, :qw])
                        for h in range(2):
                            ss_ps = psC.tile([1, FDIM], F32, tag="ssq")
                            nc.tensor.matmul(ss_ps[:, :qw],
                                             ones_p[h * HD:(h + 1) * HD, :],
                                             qsq[h * HD:(h + 1) * HD, :qw],
                                             start=True, stop=True)
                            sq = sbC.tile([1, FDIM], F32, tag="sqq")
                            # 8*sqrt(ss) = sqrt(64*ss): folds the 1/sqrt(hd) scale
                            nc.scalar.activation(sq[:, :qw], ss_ps[:, :qw],
                                                 mybir.ActivationFunctionType.Sqrt,
                                                 bias=eps_l2[:1, :1], scale=64.0)
                            rec = sbC.tile([1, FDIM], F32, tag="recq")
                            nc.vector.reciprocal(rec[:, :qw], sq[:, :qw])
                            bc_ps = psC.tile([HD, FDIM], F32, tag="bcq")
                            nc.tensor.matmul(bc_ps[:, :qw], ones_f[:, 0:HD],
                                             rec[:, :qw], start=True, stop=True)
                            nc.vector.tensor_mul(
                                qhT[h * HD:(h + 1) * HD, p, q0:q0 + qw],
                                qtmp[h * HD:(h + 1) * HD, :qw], bc_ps[:, :qw])

            xnp.__exit__(None, None, None)

            # ================= Phase D: attention (pair-outer) ==============
            with tc.tile_pool(name="pds", bufs=3) as sbD, \
                 tc.tile_pool(name="pdp_s", bufs=2, space="PSUM") as psDs, \
                 tc.tile_pool(name="pdp_a", bufs=1, space="PSUM") as psDa, \
                 tc.tile_pool(name="pdp_b", bufs=1, space="PSUM") as psDb:
                for p in range(2):
                    for qb in range(5):
                        q0 = sum(QBS[:qb])
                        qw = QBS[qb]
                        nk = nkv(qb)
                        av_ps = psDa.tile([P, 2, FDIM], F32, tag="av")
                        for kt in range(nk):
                            kw = kvw(kt)
                            s_ps = psDs.tile([P, 2, FDIM], F32, tag="sps")
                            for h in range(2):
                                nc.tensor.matmul(
                                    s_ps[0:kw, h, :qw],
                                    khT[h * HD:(h + 1) * HD, kt * P:kt * P + kw],
                                    qhT[h * HD:(h + 1) * HD, p, q0:q0 + qw],
                                    start=True, stop=True)
                            dd = kt - (QSTART // P) - 4 * qb
                            if dd >= 0:
                                for h in range(2):
                                    nc.vector.tensor_add(
                                        s_ps[0:kw, h, :qw], s_ps[0:kw, h, :qw],
                                        mask_sb[0:kw, dd, :qw])
                            ex = sbD.tile([P, 2, FDIM], BF16, tag="ex")
                            nc.scalar.activation(ex[0:kw, :, :qw], s_ps[0:kw, :, :qw],
                                                 mybir.ActivationFunctionType.Exp,
                                                 bias=0.0, scale=1.0)
                            for h in range(2):
                                nc.tensor.matmul(
                                    av_ps[0:HD + 1, h, :qw], v_aug[0:kw, kt, 0:HD + 1],
                                    ex[0:kw, h, :qw],
                                    start=(kt == 0), stop=(kt == nk - 1))
                        for h in range(2):
                            rec = sbD.tile([1, FDIM], F32, tag="recd")
                            nc.vector.reciprocal(rec[:, :qw], av_ps[HD:HD + 1, h, :qw])
                            bc_ps = psDb.tile([HD, FDIM], F32, tag="bcd")
                            nc.tensor.matmul(bc_ps[:, :qw], ones_f[:, 0:HD],
                                             rec[:, :qw], start=True, stop=True)
                            avs = sbD.tile([HD, FDIM], BF16, tag="avs")
                            nc.vector.tensor_copy(avs[:, :qw], av_ps[0:HD, h, :qw])
                            nc.vector.tensor_mul(
                                attnT[h * HD:(h + 1) * HD, p, q0:q0 + qw],
                                avs[:, :qw], bc_ps[:, :qw])
                    # reshard this head-pair while the next one computes
                    nc.sync.dma_start(gin[p][:], attnT[:, p, :])
                    if sim:
                        for r in range(4):
                            nc.sync.dma_start(gout[p][r], gin[p][:])
                    else:
                        nc.gpsimd.collective_compute(
                            "AllGather", mybir.AluOpType.bypass,
                            ins=[gin[p][:].opt()], outs=[gout[p][:].opt()],
                            replica_groups=[[0, 1, 2, 3], [4, 5, 6, 7]])

            # ================= Phase E: o-proj + residual + MLP =============
            with tc.tile_pool(name="pes", bufs=3) as sbE, \
                 tc.tile_pool(name="pew", bufs=3) as sbW, \
                 tc.tile_pool(name="per", bufs=1) as resE, \
                 tc.tile_pool(name="pep", bufs=2, space="PSUM") as psE, \
                 tc.tile_pool(name="pep1", bufs=1, space="PSUM") as psE1:
                att_sb = resE.tile([P, DCH, QS], BF16)
                for c in range(DCH):
                    nc.gpsimd.dma_start(
                        att_sb[:, c, :], gout[c % 2][c // 2][:, ds(qoff, QS)])
                xslab = resE.tile([P, DCH, QS], F32)
                nc.sync.dma_start(xslab[:],
                                  xslab_d.rearrange("(c p) t -> p c t", p=P))
                xnew = resE.tile([P, DCH, QS], F32)
                xnn = resE.tile([P, DCH, QS], BF16)
                hT = resE.tile([P, 32, QS], BF16)

                # o-proj + softcap + residual
                for dc in range(DCH):
                    for qf in range(2):
                        o_ps = psE.tile([P, QF], F32, tag="o")
                        for c in range(DCH):
                            nc.tensor.matmul(
                                o_ps[:], wo_sb[:, c, dc * P:(dc + 1) * P],
                                att_sb[:, c, qf * QF:(qf + 1) * QF],
                                start=(c == 0), stop=(c == DCH - 1))
                        th = sbE.tile([P, QF], F32, tag="th")
                        nc.scalar.activation(th[:], o_ps[:],
                                             mybir.ActivationFunctionType.Tanh,
                                             bias=0.0, scale=1.0 / SOFTCAP)
                        t15 = sbE.tile([P, QF], F32, tag="t15")
                        nc.vector.tensor_scalar_mul(t15[:], th[:], SOFTCAP)
                        nc.vector.tensor_add(xnew[:, dc, qf * QF:(qf + 1) * QF],
                                             t15[:], xslab[:, dc, qf * QF:(qf + 1) * QF])

                # rms-norm of xnew (ones-matmul over partitions trick)
                xsq = resE.tile([P, DCH, QS], BF16)
                nc.scalar.square(xsq[:], xnew[:])
                for qf in range(2):
                    ss_ps = psE1.tile([1, QF], F32, tag="ssx")
                    for c in range(DCH):
                        nc.tensor.matmul(ss_ps[:], ones_p[:],
                                         xsq[:, c, qf * QF:(qf + 1) * QF],
                                         start=(c == 0), stop=(c == DCH - 1))
                    sq = sbE.tile([1, QF], F32, tag="sqx")
                    nc.scalar.activation(sq[:], ss_ps[:],
                                         mybir.ActivationFunctionType.Sqrt,
                                         bias=eps_x[:1, :1], scale=1.0 / D)
                    rec = sbE.tile([1, QF], F32, tag="recx")
                    nc.vector.reciprocal(rec[:], sq[:])
                    bc_ps = psE1.tile([P, QF], F32, tag="bcx")
                    nc.tensor.matmul(bc_ps[:], ones_f[:], rec[:],
                                     start=True, stop=True)
                    for c in range(DCH):
                        nc.vector.tensor_mul(xnn[:, c, qf * QF:(qf + 1) * QF],
                                             xnew[:, c, qf * QF:(qf + 1) * QF],
                                             bc_ps[:])

                # fc + relu^2
                for hc in range(32):
                    wfc_t = sbW.tile([P, DCH, P], BF16, tag="wfc")
                    nc.sync.dma_start(wfc_t[:],
                                      wfc_d[hc].rearrange("(c p) f -> p c f", p=P))
                    for qf in range(2):
                        h_ps = psE.tile([P, QF], F32, tag="h")
                        for c in range(DCH):
                            nc.tensor.matmul(h_ps[:], wfc_t[:, c, :],
                                             xnn[:, c, qf * QF:(qf + 1) * QF],
                                             start=(c == 0), stop=(c == DCH - 1))
                        hr = sbE.tile([P, QF], BF16, tag="hr")
                        nc.scalar.activation(hr[:], h_ps[:],
                                             mybir.ActivationFunctionType.Relu,
                                             bias=0.0, scale=1.0)
                        nc.vector.tensor_mul(hT[:, hc, qf * QF:(qf + 1) * QF],
                                             hr[:], hr[:])

                # proj + residual + out
                for dc in range(DCH):
                    wpr_t = sbW.tile([P, 32, P], BF16, tag="wpr")
                    nc.sync.dma_start(wpr_t[:],
                                      wproj_d[dc].rearrange("(c p) f -> p c f", p=P))
                    for qf in range(2):
                        pr_ps = psE.tile([P, QF], F32, tag="pr")
                        for c in range(32):
                            nc.tensor.matmul(pr_ps[:], wpr_t[:, c, :],
                                             hT[:, c, qf * QF:(qf + 1) * QF],
                                             start=(c == 0), stop=(c == 31))
                        ot = sbE.tile([P, QF], F32, tag="ot")
                        nc.vector.tensor_add(ot[:], pr_ps[:],
                                             xnew[:, dc, qf * QF:(qf + 1) * QF])
                        nc.sync.dma_start(
                            out_d.rearrange("(c p) t -> p c t", p=P)[:, dc, qf * QF:(qf + 1) * QF],
                            ot[:])

    nc.compile()
    return nc


_NC_CACHE = None


def _get_nc():
    global _NC_CACHE
    if _NC_CACHE is None:
        _NC_CACHE = build()
    return _NC_CACHE


def _bf16(a):
    return a.astype(ml_dtypes.bfloat16)


def make_in_maps(x, Wq, Wk, Wv, Wo, Wfc, Wproj):
    ms = np.float32(1.0) / np.sqrt(np.mean(x.astype(np.float32) ** 2, axis=-1,
                                           keepdims=True) + EPS_RMS)
    xn = (x * ms).astype(np.float32)

    mask = np.zeros((4, P, FDIM), np.float32)
    ii = np.arange(P)[:, None]
    jj = np.arange(FDIM)[None, :]
    for d in range(4):
        mask[d] = np.where(ii + 128 * d <= jj, 0.0, NEG)

    wfc_t = np.ascontiguousarray(
        _bf16(Wfc.T).reshape(D, 32, P).transpose(1, 0, 2))       # [32, D, 128]
    wpr_t = np.ascontiguousarray(
        _bf16(Wproj.T).reshape(4 * D, DCH, P).transpose(1, 0, 2))  # [8, 4D, 128]
    wo_t = np.ascontiguousarray(_bf16(Wo.T))
    ones_p = np.ones((P, 1), ml_dtypes.bfloat16)
    ones_f = np.ones((1, P), np.float32)
    ident = np.eye(P, dtype=ml_dtypes.bfloat16)

    in_maps = []
    for core in range(8):
        b, g = core // NG, core % NG
        xnt = np.ascontiguousarray(_bf16(xn[b].T))
        xslab = np.ascontiguousarray(
            x[b, QSTART + g * QS:QSTART + (g + 1) * QS, :].T.astype(np.float32))
        wq = np.ascontiguousarray(_bf16(Wq.T[:, g * 4 * HD:(g + 1) * 4 * HD]))
        wkv = np.ascontiguousarray(_bf16(np.concatenate(
            [Wk.T[:, g * HD:(g + 1) * HD], Wv.T[:, g * HD:(g + 1) * HD]], axis=1)))
        in_maps.append({
            "xnt": xnt, "xslab": xslab, "wq": wq, "wkv": wkv, "wo": wo_t,
            "wfc": wfc_t, "wproj": wpr_t, "mask": mask,
            "qoff": np.array([[g * QS]], np.int32),
            "ones_p": ones_p, "ones_f": ones_f, "ident": ident,
        })
    return in_maps


def kernel(x, Wq, Wk, Wv, Wo, Wfc, Wproj, chunk_start_idx, chunk_len,
           n_scratchpad, _trace=False, _tmpdir=None):
    assert x.shape == (B, T, D) and chunk_start_idx == QSTART
    nc = _get_nc()
    in_maps = make_in_maps(x, Wq, Wk, Wv, Wo, Wfc, Wproj)
    kwargs = {}
    if _trace:
        kwargs = dict(trace=True, tmpdir=_tmpdir)
    res = run_bass_kernel_spmd(nc, in_maps, core_ids=list(range(8)), **kwargs)
    out = np.empty((B, T, D), np.float32)
    out[:, :QSTART] = x[:, :QSTART]
    for core in range(8):
        b, g = core // NG, core % NG
        out[b, QSTART + g * QS:QSTART + (g + 1) * QS] = res.results[core]["out"].T
    if _trace:
        return out, res
    return out

